# revision 4
# baseline (speedup 1.0000x reference)
"""Trainium2 Bass kernel for nn_DirectedSpatialConv (gnn_message_passing), v3.

out[b,o,n,t] = sum_k W[k] @_c ( Abar_k @_j x[b] + (Pe_k+Ce_k) @_e x_other[b] ) + bias

Sharding: data-parallel over batch B=8, one batch element per NeuronCore.

v3: stage1 in fp8e4 (e4m3) DoubleRow perf mode — 2 PE instructions per
t-pair instead of 3 bf16 ones, at half the per-column cost. Precision is
recovered by mean-removing Qe = Pe+Ce over e (Qe = mu + V): the x8/abar8
and xo8/V8 products quantize fine (abar is degree-normalized so the x path
is numerically tiny), and the dominant rank-1 term mu[k,n]*S[c,t]
(S = sum_e x_other) is carried exactly by 4 extra fp8 contraction rows
(S0,S1 x mu0,dmu splits). Host-measured gate error: 1.39e-2 (< 2e-2).

Time steps processed in pairs t=(2*tp+dt):
  stage1 (PE, fp8 DR):
      DR_A: lhsT = [x8 | aux] [j, 2, (dt,c)], rhs = [abar8 | auxW] [j, 2, (k n)]
      DR_B: lhsT = [xo8_h0 | xo8_h1],         rhs = [V8_h0 | V8_h1]
      -> psum agg16[(dt,c), (k,n)] (= 16x true agg; /16 folded into wblk)
  copy  (ACT/DVE):  psum -> sbuf agg ring (bf16)
  stage2 (PE, bf16): lhsT = blkdiag(Wk^T,Wk^T)/16, rhs = agg -> psum out
  copy  (ACT):  psum + bias -> sbuf bf16 -> DMA out[dt,o,tp,n]
"""

import sys

sys.path.insert(0, "/opt/trn_rl_repo")

import os
import numpy as np
import ml_dtypes
from contextlib import ExitStack

import concourse.bass as bass
import concourse.tile as tile
from concourse import bacc, mybir
from concourse.bass_utils import run_bass_kernel_spmd

# Problem shapes (hardcoded per contest contract).
B, C, N, E, T, K = 8, 64, 128, 256, 1024, 4
O = 64  # output channels
EPS = 1e-3
N_CORES = 8
SCALE = 16.0  # agg scale carried by fp8 weights; /SCALE folded into wblk

TP = T // 2        # 512 t-pairs
TPS = 64           # t-pairs per slab
NSLAB = TP // TPS  # 8
GP = 8             # t-pairs per group (stage2 granularity)
NG = TPS // GP     # groups per slab

BF16 = mybir.dt.bfloat16
F32 = mybir.dt.float32
F8 = mybir.dt.float8e4
F8NP = ml_dtypes.float8_e4m3
DR = mybir.MatmulPerfMode.DoubleRow

_compiled = None


def _env(k, d):
    return int(os.environ.get(k, d))


def _build(repeat=1):
    nc = bacc.Bacc("TRN2", target_bir_lowering=False, debug=False,
                   num_devices=N_CORES)

    # host-prepped layouts:
    #   xw8 [j, tp, dt, c]        = e4m3(x[c, j, 2tp+dt])
    #   auxd[r, tp, dt, c]        = {S0,S1,S0,S1}[r][c, 2tp+dt]
    #   xow8[e_loc, tp, h, dt*64+c] = e4m3(xo[c, 128h+e_loc, 2tp+dt])
    #   wa  [j, 2, k, n]          = [abar8 | auxW]
    #   wb  [e_loc, 2, k, n]      = [V8_h0 | V8_h1]
    #   wblk[(dt,c), k, (dt',o)]  = (dt==dt') W[k, o, c]/SCALE
    xw8_d = nc.dram_tensor("xw8", [N, TP, 2, C], F8, kind="ExternalInput").ap()
    aux_d = nc.dram_tensor("auxd", [4, TP, 2, C], F8, kind="ExternalInput").ap()
    xow8_d = nc.dram_tensor("xow8", [128, TP, 2, 2 * C], F8,
                            kind="ExternalInput").ap()
    wa_d = nc.dram_tensor("wa", [N, 2, K, N], F8, kind="ExternalInput").ap()
    wb_d = nc.dram_tensor("wb", [128, 2, K, N], F8, kind="ExternalInput").ap()
    wblk_d = nc.dram_tensor("wblk", [128, K, 128], BF16, kind="ExternalInput").ap()
    bias_d = nc.dram_tensor("biasv", [128, 1], F32, kind="ExternalInput").ap()
    # out[dt, o, tp, n] bf16; host reshapes to [o, n, t] and casts f32
    out_d = nc.dram_tensor("out", [2, O, TP, N], BF16, kind="ExternalOutput").ap()

    with tile.TileContext(nc) as tc, ExitStack() as ctx:
        consts = ctx.enter_context(tc.tile_pool(name="consts", bufs=1))
        px = ctx.enter_context(tc.tile_pool(name="px", bufs=_env("KB_PX", 2)))
        pxo = ctx.enter_context(tc.tile_pool(name="pxo", bufs=_env("KB_PXO", 2)))
        pagg = ctx.enter_context(tc.tile_pool(name="pagg", bufs=_env("KB_PAGG", 2)))
        pout = ctx.enter_context(tc.tile_pool(name="pout", bufs=_env("KB_POUT", 2)))
        ps1 = ctx.enter_context(
            tc.tile_pool(name="ps1", bufs=_env("KB_PS1", 4), space="PSUM"))
        ps2 = ctx.enter_context(
            tc.tile_pool(name="ps2", bufs=_env("KB_PS2", 4), space="PSUM"))

        wa_sb = consts.tile([N, 2, K, N], F8)
        nc.sync.dma_start(wa_sb[:], wa_d)
        wb_sb = consts.tile([128, 2, K, N], F8)
        nc.sync.dma_start(wb_sb[:], wb_d)
        wblk_sb = consts.tile([128, K, 128], BF16)
        nc.sync.dma_start(wblk_sb[:], wblk_d)
        bias_sb = consts.tile([128, 1], F32)
        nc.sync.dma_start(bias_sb[:], bias_d)

        # which t-pairs of each group get their agg copy on ACT vs DVE
        act_set = set(
            int(c) for c in os.environ.get("KB_ACT_COPIES", "36").strip()
            if c.strip())

        rep_ctx = tc.For_i(0, repeat, 1) if repeat > 1 else None
        if rep_ctx is not None:
            ctx.enter_context(rep_ctx)

        xw_tiles = {}
        xow_tiles = {}

        def emit_loads(s):
            # chunked by group so group g's matmuls only wait on chunk g
            tp0 = s * TPS
            xt = px.tile([N, TPS, 2, 2 * C], F8, tag="xw")
            xw_tiles[s] = xt
            xot = pxo.tile([128, TPS, 2, 2 * C], F8, tag="xow")
            xow_tiles[s] = xot
            if s < _env("KB_PX", 2):
                # one-time zero of the aux tile's unused contraction rows
                # (rows 4-127 of ktile 1); zero weights there make the
                # product exact, but junk fp8 bytes could be NaN.
                nc.gpsimd.memset(xt[:, :, 1, :], 0)
            for g in range(NG):
                a, b = g * GP, (g + 1) * GP
                nc.gpsimd.dma_start(xt[:, a:b, 0, :],
                                    xw8_d[:, tp0 + a:tp0 + b, :, :])
                nc.gpsimd.dma_start(xt[0:4, a:b, 1, :],
                                    aux_d[:, tp0 + a:tp0 + b, :, :])
                nc.gpsimd.dma_start(xot[:, a:b, :, :],
                                    xow8_d[:, tp0 + a:tp0 + b, :, :])

        def emit_stage1(s, g, agg):
            """One group: GP t-pairs -> agg sbuf tile [128, GP, K, N] bf16."""
            xt = xw_tiles[s]
            xot = xow_tiles[s]
            for i in range(GP):
                tpl = g * GP + i
                psA = ps1.tile([128, K, N], F32, tag="s1")
                nc.tensor.matmul(psA[:], xt[:, tpl, :, :],
                                 wa_sb.rearrange("j t k n -> j t (k n)"),
                                 start=True, stop=False, perf_mode=DR)
                nc.tensor.matmul(psA[:], xot[:, tpl, :, :],
                                 wb_sb.rearrange("e t k n -> e t (k n)"),
                                 start=False, stop=True, perf_mode=DR)
                if i in act_set:
                    nc.scalar.activation(agg[:, i], psA[:],
                                         mybir.ActivationFunctionType.Copy)
                else:
                    nc.vector.tensor_copy(out=agg[:, i], in_=psA[:])

        def emit_stage2(s, g, agg):
            """Group g: agg [128,(GP,K,N)] -> out psum [(dt,o),(tb,n)] -> sbuf+DMA."""
            tp0 = s * TPS + g * GP
            ot = pout.tile([128, GP, N], BF16, tag="out")
            for half in range(2):
                psB = ps2.tile([128, GP // 2, N], F32, tag="s2")
                hs = slice(half * (GP // 2), (half + 1) * (GP // 2))
                for k in range(K):
                    nc.tensor.matmul(
                        psB[:],
                        wblk_sb[:, k, :],
                        agg[:, hs, k, :],
                        start=(k == 0), stop=(k == K - 1))
                nc.scalar.activation(
                    ot[:, hs, :],
                    psB[:],
                    mybir.ActivationFunctionType.Identity,
                    bias=bias_sb[:])
                # DMA per half: shortens the end-of-kernel drain chain
                h0 = tp0 + half * (GP // 2)
                nc.sync.dma_start(
                    out_d[:, :, h0:h0 + GP // 2, :].rearrange(
                        "d o p n -> (d o) p n"),
                    ot[:, hs, :])

        # emission: slab loads up front; stage2 lags stage1 by one group
        prev = None
        for s in range(NSLAB):
            emit_loads(s)
            for g in range(NG):
                agg = pagg.tile([128, GP, K, N], BF16, tag="agg")
                emit_stage1(s, g, agg)
                if prev is not None:
                    emit_stage2(*prev)
                prev = (s, g, agg)
        emit_stage2(*prev)

    nc.compile()
    return nc


def _f8(a):
    return np.asarray(a, np.float32).astype(F8NP)


def _prep_consts(Av, Pe, Ce, W, bias):
    Av = np.asarray(Av, np.float64)
    dis = 1.0 / np.sqrt(Av.sum(-1) + EPS)           # [K, N]
    abar = dis[:, :, None] * Av * dis[:, None, :]   # [K, n, j]
    abar8 = _f8(SCALE * abar.transpose(2, 0, 1))    # [j, k, n]

    Qe = np.asarray(Pe, np.float64) + np.asarray(Ce, np.float64)  # [K, n, e]
    mu = Qe.mean(-1)                                # [K, n]
    V = Qe - mu[..., None]                          # [K, n, e]
    mu0 = _f8(SCALE * mu)
    dmu = _f8(SCALE * mu - mu0.astype(np.float64))
    v8 = _f8(SCALE * V.transpose(2, 0, 1))          # [e, k, n]

    wa = np.zeros((N, 2, K, N), F8NP)
    wa[:, 0] = abar8
    wa[0, 1], wa[1, 1] = mu0, mu0
    wa[2, 1], wa[3, 1] = dmu, dmu

    wb = np.ascontiguousarray(
        v8.reshape(2, 128, K, N).transpose(1, 0, 2, 3))  # [e_loc, h, k, n]

    W = np.asarray(W, np.float64)                    # [K, o, c]
    wblk = np.zeros((128, K, 128), np.float64)
    for dt in range(2):
        for k in range(K):
            wblk[dt * 64:(dt + 1) * 64, k, dt * 64:(dt + 1) * 64] = \
                W[k].T / SCALE
    wblk_in = wblk.astype(ml_dtypes.bfloat16)

    bsum = np.asarray(bias, np.float64).sum(0)       # [O]
    biasv = np.tile(bsum, 2).reshape(128, 1).astype(np.float32)
    return wa, wb, wblk_in, biasv


def _make_in_maps(inp_np):
    x = np.asarray(inp_np["x"], np.float32)
    xo = np.asarray(inp_np["x_other"], np.float32)
    wa, wb, wblk_in, biasv = _prep_consts(
        inp_np["Av"], inp_np["Pe"], inp_np["Ce"], inp_np["W"], inp_np["bias"])
    # xw8[j, tp, dt, c] = e4m3(x[c, j, t])
    xw8 = _f8(np.ascontiguousarray(
        x.transpose(0, 2, 3, 1).reshape(B, N, TP, 2, C)))
    # xow8[e_loc, tp, h, (dt,c)] = e4m3(xo[c, e, t])
    xow8 = _f8(np.ascontiguousarray(
        xo.transpose(0, 2, 3, 1).reshape(B, 2, 128, TP, 2 * C)
        .transpose(0, 2, 3, 1, 4)))
    # S = sum_e xo[c, e, t]; split into two e4m3 layers
    S = np.asarray(xo, np.float64).sum(2)            # [B, C, T]
    S0 = _f8(S)
    S1 = _f8(S - S0.astype(np.float64))
    aux = np.zeros((B, 4, C, T), F8NP)
    aux[:, 0], aux[:, 1], aux[:, 2], aux[:, 3] = S0, S1, S0, S1
    aux = np.ascontiguousarray(
        aux.transpose(0, 1, 3, 2).reshape(B, 4, TP, 2, C))
    in_maps = []
    for b in range(N_CORES):
        in_maps.append({
            "xw8": xw8[b],
            "auxd": aux[b],
            "xow8": xow8[b],
            "wa": wa,
            "wb": wb,
            "wblk": wblk_in,
            "biasv": biasv,
        })
    return in_maps


def kernel(x, x_other, Av, Pe, Ce, W, bias):
    global _compiled
    if _compiled is None:
        _compiled = _build()
    nc = _compiled

    in_maps = _make_in_maps(dict(x=x, x_other=x_other, Av=Av, Pe=Pe, Ce=Ce,
                                 W=W, bias=bias))
    res = run_bass_kernel_spmd(nc, in_maps, core_ids=list(range(N_CORES)))
    outs = []
    for b in range(N_CORES):
        o = np.asarray(res.results[b]["out"], np.float32)  # [2, O, TP, N]
        outs.append(o.transpose(1, 3, 2, 0).reshape(O, N, T))
    return np.stack(outs).astype(np.float32)
